# revision 2
# baseline (speedup 1.0000x reference)
"""Bass/Trainium2 kernel for nn_CrossAttentionLayer — v2 (SBUF-resident, bf16).

out = softmax((x_q Wq^T + bq)(x_k Wk^T + bk)^T) (x_v Wv^T + bv)

Sharding: data-parallel over batch B=8 across the 8 NeuronCores.

Design (vs v1 baseline):
  - All intermediates (k^T, v, q^T, exp(scores)) stay resident in SBUF in
    bf16 — no DRAM round-trips for v/q^T and no 512-byte-chunk descriptor
    floods. Every DRAM tensor moves in 1-2 DMAs with 8-32 KB contiguous
    per-partition descriptors (host packs partition-major layouts).
  - bk drops out of softmax (per-row constant). bv is folded into v on
    device (rows of normalized attn sum to 1). Softmax normalization is
    applied on device: pv^T blocks are PE-transposed to natural [q, d]
    layout and multiplied by 1/rowsum (per-partition scalar), so the host
    does no math beyond a layout unpack + f32 cast.
  - Scores are computed transposed ([key, query]) so PV consumes v in its
    natural [key, d] layout; rowsums over the key (partition) axis come
    from a ones-vector matmul on the PE; the [1,512] rowsum row is
    transposed to per-partition [128,4] via four 1-row matmuls.
"""

import sys

if "/opt/trn_rl_repo" not in sys.path:
    sys.path.insert(0, "/opt/trn_rl_repo")

import numpy as np

B = 8          # batch == number of cores
D = 1024       # model/latent dim
N = 2048       # tokens (queries == keys)
P = 128        # partitions
DC = D // P    # 8 chunks of the d/e axis
JT = N // P    # 16 key tiles
F = 512        # matmul moving free dim (PSUM bank)
NB = N // F    # 4 query blocks
NH = 2         # x input halves (n-dim)

_CACHE = {}


def _build_nc():
    import concourse.bass as bass
    import concourse.mybir as mybir
    import concourse.tile as tile
    from concourse import bacc
    from concourse.masks import make_identity
    from contextlib import ExitStack

    f32 = mybir.dt.float32
    bf16 = mybir.dt.bfloat16
    fp16 = mybir.dt.float16
    EXP = mybir.ActivationFunctionType.Exp

    nc = bacc.Bacc("TRN2", target_bir_lowering=False, debug=False, num_devices=B)

    # Packed DRAM layouts (host side prepares these):
    #   x*  [P, DC, N]  : x^T chunked  — x*[p, c, n] = x[n, c*P + p]
    #   w*  [P, DC, D]  : W^T chunked  — w*[p, c, e] = W[e, c*P + p]
    #   bq  [P, DC]     : bq[p, c] = bias_q[c*P + p]
    #   bv  [1, D]
    #   out [P, NB*JT//4? -> N//P blocks, D] : out[p, blk, d] = y[blk*P + p, d]
    xq = nc.dram_tensor("xq", [P, DC, N], fp16, kind="ExternalInput").ap()
    xk = nc.dram_tensor("xk", [P, DC, N], fp16, kind="ExternalInput").ap()
    xv = nc.dram_tensor("xv", [P, DC, N], bf16, kind="ExternalInput").ap()
    wq = nc.dram_tensor("wq", [P, DC, D], fp16, kind="ExternalInput").ap()
    wk = nc.dram_tensor("wk", [P, DC, D], fp16, kind="ExternalInput").ap()
    wv = nc.dram_tensor("wv", [P, DC, D], bf16, kind="ExternalInput").ap()
    bq = nc.dram_tensor("bq", [P, DC], f32, kind="ExternalInput").ap()
    bv = nc.dram_tensor("bv", [1, D], f32, kind="ExternalInput").ap()

    outp = nc.dram_tensor("outp", [P, N // P, D], bf16, kind="ExternalOutput").ap()

    HN = N // NH  # tokens per x half

    with ExitStack() as ctx:
        tc = ctx.enter_context(tile.TileContext(nc))
        res = ctx.enter_context(tc.tile_pool(name="res", bufs=1))
        xst = ctx.enter_context(tc.tile_pool(name="xst", bufs=2))
        wst = ctx.enter_context(tc.tile_pool(name="wst", bufs=2))
        exp_ = ctx.enter_context(tc.tile_pool(name="exp", bufs=1))
        outs = ctx.enter_context(tc.tile_pool(name="outs", bufs=2))
        sml = ctx.enter_context(tc.tile_pool(name="sml", bufs=1))
        smr = ctx.enter_context(tc.tile_pool(name="smr", bufs=2))
        psa = ctx.enter_context(tc.tile_pool(name="psa", bufs=3, space="PSUM"))
        pss = ctx.enter_context(tc.tile_pool(name="pss", bufs=2, space="PSUM"))
        psr = ctx.enter_context(tc.tile_pool(name="psr", bufs=1, space="PSUM"))
        pst = ctx.enter_context(tc.tile_pool(name="pst", bufs=2, space="PSUM"))

        # ---- constants / small tensors ----
        ident = sml.tile([P, P], bf16, name="ident", tag="c_id")
        make_identity(nc, ident)
        ones_b = sml.tile([P, 1], bf16, name="ones_b", tag="c_ones")
        nc.vector.memset(ones_b, 1.0)
        one1 = sml.tile([1, 1], f32, name="one1", tag="c_one1")
        nc.vector.memset(one1, 1.0)
        bq_sb = sml.tile([P, DC], f32, name="bq_sb", tag="c_bq")
        nc.sync.dma_start(out=bq_sb, in_=bq)
        bv_sb = sml.tile([1, D], f32, name="bv_sb", tag="c_bv")
        nc.sync.dma_start(out=bv_sb, in_=bv)
        ones_row = sml.tile([1, P], f32, name="ones_row", tag="c_onesr")
        nc.vector.memset(ones_row, 1.0)
        # bv broadcast across partitions: bvb[p, e] = bv[e]
        bv_bc = sml.tile([P, D], f32, name="bv_bc", tag="c_bvb")
        for h in range(2):
            pb = pst.tile([P, F], f32, name="pb", tag="pst")
            nc.tensor.matmul(
                pb, lhsT=ones_row, rhs=bv_sb[:, h * F:(h + 1) * F],
                start=True, stop=True,
            )
            nc.vector.tensor_copy(bv_bc[:, h * F:(h + 1) * F], pb)

        # ---- resident tiles ----
        kt = res.tile([P, DC, N], fp16, name="kt", tag="kt")      # k^T[e, n]
        vv = res.tile([P, JT, D], bf16, name="vv", tag="vv")      # v'[n, e]
        qt = res.tile([P, DC, N], fp16, name="qt", tag="qt")      # q^T[e, n]

        # ---- Phase 1: projections (k^T, v', q^T), x loaded in halves ----
        def proj_kq(dst, xsrc, wsrc, is_q):
            w_sb = wst.tile([P, DC, D], fp16, name="w_sb", tag="w")
            nc.sync.dma_start(out=w_sb, in_=wsrc)
            for h in range(NH):
                xh = xst.tile([P, DC, HN], fp16, name="xh", tag="xj")
                nc.sync.dma_start(out=xh, in_=xsrc[:, :, h * HN:(h + 1) * HN])
                for c in range(DC):
                    for nb in range(HN // F):
                        ps = psa.tile([P, F], f32, name="ps_p", tag="psa")
                        for dp in range(DC):
                            nc.tensor.matmul(
                                ps,
                                lhsT=w_sb[:, dp, c * P:(c + 1) * P],
                                rhs=xh[:, dp, nb * F:(nb + 1) * F],
                                start=(dp == 0),
                                stop=(dp == DC - 1),
                            )
                        dsl = dst[:, c, h * HN + nb * F:h * HN + (nb + 1) * F]
                        if is_q:
                            nc.vector.tensor_scalar_add(dsl, ps, bq_sb[:, c:c + 1])
                        else:
                            nc.scalar.copy(dsl, ps)

        def proj_v():
            w_sb = wst.tile([P, DC, D], bf16, name="w_sb", tag="w")
            nc.sync.dma_start(out=w_sb, in_=wv)
            for h in range(NH):
                xh = xst.tile([P, DC, HN], bf16, name="xh", tag="xj")
                nc.sync.dma_start(out=xh, in_=xv[:, :, h * HN:(h + 1) * HN])
                for jt in range(HN // P):
                    jg = h * (HN // P) + jt
                    for eh in range(D // F):
                        ps = psa.tile([P, F], f32, name="ps_v", tag="psa")
                        for dp in range(DC):
                            nc.tensor.matmul(
                                ps,
                                lhsT=xh[:, dp, jt * P:(jt + 1) * P],
                                rhs=w_sb[:, dp, eh * F:(eh + 1) * F],
                                start=(dp == 0),
                                stop=(dp == DC - 1),
                            )
                        nc.vector.tensor_tensor(
                            out=vv[:, jg, eh * F:(eh + 1) * F],
                            in0=ps,
                            in1=bv_bc[:, eh * F:(eh + 1) * F],
                            op=mybir.AluOpType.add,
                        )

        proj_kq(kt, xk, wk, is_q=False)
        proj_v()
        proj_kq(qt, xq, wq, is_q=True)

        # ---- Phase 2: per 512-query block: scores^T, exp, rowsum, PV ----
        for t in range(NB):
            ex = exp_.tile([P, JT, F], bf16, name="ex", tag="ex")
            rp = psr.tile([1, F], f32, name="rp", tag="psr")
            # scores + exp, with the rowsum matmul software-pipelined one jt
            # behind so the PE never waits on the Act engine's exp output.
            for jt in range(JT):
                ps = pss.tile([P, F], f32, name="ps_s", tag="pss")
                for c in range(DC):
                    nc.tensor.matmul(
                        ps,
                        lhsT=kt[:, c, jt * P:(jt + 1) * P],
                        rhs=qt[:, c, t * F:(t + 1) * F],
                        start=(c == 0),
                        stop=(c == DC - 1),
                    )
                nc.scalar.activation(ex[:, jt, :], ps, EXP)
                if jt > 0:
                    nc.tensor.matmul(
                        rp,
                        lhsT=ones_b,
                        rhs=ex[:, jt - 1, :],
                        start=(jt - 1 == 0),
                        stop=False,
                        skip_group_check=True,
                    )
            nc.tensor.matmul(
                rp,
                lhsT=ones_b,
                rhs=ex[:, JT - 1, :],
                start=False,
                stop=True,
                skip_group_check=True,
            )
            rs = smr.tile([1, F], f32, name="rs", tag="rs")
            nc.vector.tensor_copy(rs, rp)

            # PV, with transpose+normalize pipelined one dc behind (so the PE
            # overlaps the Act engine's psum->sbuf copy), and the rowsum
            # transpose/reciprocal slotted in after PV(0).
            ot = outs.tile([P, NB, D], bf16, name="ot", tag="ot")
            rinv = None
            pvs_q = []
            for dc in range(DC):
                pv = psa.tile([P, F], f32, name="pv", tag="psa")
                for jt in range(JT):
                    nc.tensor.matmul(
                        pv,
                        lhsT=vv[:, jt, dc * P:(dc + 1) * P],
                        rhs=ex[:, jt, :],
                        start=(jt == 0),
                        stop=(jt == JT - 1),
                    )
                pvs = smr.tile([P, F], bf16, name="pvs", tag="pvs")
                nc.scalar.copy(pvs, pv)
                pvs_q.append(pvs)
                if dc == 0:
                    # rowsum row -> per-partition reciprocal [P, NB]
                    rpt = pst.tile([P, NB], f32, name="rpt", tag="pst")
                    for qb in range(NB):
                        nc.tensor.matmul(
                            rpt[:, qb:qb + 1],
                            lhsT=rs[0:1, qb * P:(qb + 1) * P],
                            rhs=one1,
                            start=True,
                            stop=True,
                            skip_group_check=True,
                        )
                    rinv = smr.tile([P, NB], f32, name="rinv", tag="rinv")
                    nc.vector.reciprocal(rinv, rpt)
                else:
                    pvp = pvs_q[dc - 1]
                    for qb in range(NB):
                        pt = pst.tile([P, P], bf16, name="pt", tag="pst")
                        nc.tensor.transpose(pt, pvp[:, qb * P:(qb + 1) * P], ident)
                        nc.vector.tensor_scalar_mul(
                            ot[:, qb, (dc - 1) * P:dc * P], pt, rinv[:, qb:qb + 1]
                        )
            pvp = pvs_q[DC - 1]
            for qb in range(NB):
                pt = pst.tile([P, P], bf16, name="pt", tag="pst")
                nc.tensor.transpose(pt, pvp[:, qb * P:(qb + 1) * P], ident)
                nc.vector.tensor_scalar_mul(
                    ot[:, qb, (DC - 1) * P:DC * P], pt, rinv[:, qb:qb + 1]
                )
            nc.sync.dma_start(out=outp[:, t * NB:(t + 1) * NB, :], in_=ot)

    nc.compile()
    return nc


def get_nc():
    if "nc" not in _CACHE:
        _CACHE["nc"] = _build_nc()
    return _CACHE["nc"]


def make_in_maps(query, key, value, Wq, bq, Wk, bk, Wv, bv):
    import ml_dtypes

    bf16 = ml_dtypes.bfloat16
    query = np.asarray(query, dtype=np.float32)
    key = np.asarray(key, dtype=np.float32)
    value = np.asarray(value, dtype=np.float32)

    def pack_w(W, dt):
        # [P, DC, D]: w[p, c, e] = W[e, c*P + p]
        Wt = np.asarray(W, dtype=np.float32).T  # [d, e]
        return np.ascontiguousarray(
            Wt.reshape(DC, P, D).transpose(1, 0, 2)
        ).astype(dt)

    def pack_x(x, dt):
        # [P, DC, N]: xp[p, c, n] = x[n, c*P + p]
        return np.ascontiguousarray(
            x.reshape(N, DC, P).transpose(2, 1, 0)
        ).astype(dt)

    wq_p = pack_w(Wq, np.float16)
    wk_p = pack_w(Wk, np.float16)
    wv_p = pack_w(Wv, bf16)
    bq_p = np.ascontiguousarray(
        np.asarray(bq, dtype=np.float32).reshape(DC, P).T
    )
    bv_p = np.ascontiguousarray(np.asarray(bv, dtype=np.float32).reshape(1, D))

    in_maps = []
    for b in range(B):
        in_maps.append(
            {
                "xq": pack_x(query[b], np.float16),
                "xk": pack_x(key[b], np.float16),
                "xv": pack_x(value[b], bf16),
                "wq": wq_p,
                "wk": wk_p,
                "wv": wv_p,
                "bq": bq_p,
                "bv": bv_p,
            }
        )
    return in_maps


def postprocess(results):
    outs = []
    for b in range(B):
        op = np.asarray(results[b]["outp"])  # [P, N//P, D] bf16
        outs.append(
            op.astype(np.float32).transpose(1, 0, 2).reshape(N, D)
        )
    return np.stack(outs)


def kernel(query, key, value, Wq, bq, Wk, bk, Wv, bv):
    from concourse.bass_utils import run_bass_kernel_spmd

    nc = get_nc()
    in_maps = make_in_maps(query, key, value, Wq, bq, Wk, bk, Wv, bv)
    res = run_bass_kernel_spmd(nc, in_maps, list(range(B)))
    return postprocess(res.results)


# revision 5
# speedup vs baseline: 32.9243x; 32.9243x over previous
"""Bass/Trainium2 kernel for nn_CrossAttentionLayer — v3 (SBUF-resident, M-folded).

out = softmax((x_q Wq^T + bq)(x_k Wk^T + bk)^T) (x_v Wv^T + bv)

Sharding: data-parallel over batch B=8 across the 8 NeuronCores.

Design:
  - All intermediates stay resident in SBUF (fp16/bf16) — no DRAM
    round-trips and no small-chunk DMA descriptor floods. Every DRAM
    tensor moves in a few DMAs with 8-32 KB contiguous per-partition
    descriptors (host packs partition-major layouts).
  - Weight folding: logits = xq (Wq^T Wk) xk^T + bq.k_j (+ per-row
    constants that cancel in softmax). The host precomputes
    M = Wq^T Wk and u = Wk^T bq in f32, so the device never computes the
    k projection at all: t1 = xq M (one projection), scores^T = xk t1^T
    with raw xk as the stationary operand, and the per-key bias
    c_j = u.xk_j enters for free through the Exp activation's bias port.
  - bk drops out of softmax (per-row constant). bv is folded into v on
    device (rows of normalized attn sum to 1). Softmax normalization is
    applied on device with a single per-partition tensor_scalar multiply:
    the PV matmul uses exp(scores^T) as the stationary operand, which
    makes its PSUM output come out directly in natural [q, d] layout (no
    transposes anywhere). The host does no math beyond a layout unpack +
    f32 cast.
  - Scores are computed transposed ([key, query]); per-query rowsums come
    out per-partition [128,1] from the same ex-stationary operand matmul'd
    against a ones vector (16 ap_size-1 matmuls per query block, ~free on
    the PE).
  - fp16 for the whole scores path (4x less logit rounding than bf16);
    bf16 for exp(scores) and v (exponent range needed there).
"""

import sys

if "/opt/trn_rl_repo" not in sys.path:
    sys.path.insert(0, "/opt/trn_rl_repo")

import numpy as np

B = 8          # batch == number of cores
D = 1024       # model/latent dim
N = 2048       # tokens (queries == keys)
P = 128        # partitions
DC = D // P    # 8 chunks of the d/e axis
JT = N // P    # 16 key tiles
F = 512        # matmul moving free dim (PSUM bank)
NB = N // F    # 4 query blocks
NH = 2         # x input halves (n-dim)

_CACHE = {}


def _build_nc():
    import concourse.bass as bass
    import concourse.mybir as mybir
    import concourse.tile as tile
    from concourse import bacc
    from contextlib import ExitStack

    f32 = mybir.dt.float32
    bf16 = mybir.dt.bfloat16
    fp16 = mybir.dt.float16
    EXP = mybir.ActivationFunctionType.Exp

    nc = bacc.Bacc("TRN2", target_bir_lowering=False, debug=False, num_devices=B)

    # Packed DRAM layouts (host side prepares these):
    #   xq/xk/xv [P, DC, N] : x^T chunked — x*[p, c, n] = x[n, c*P + p]
    #   mm       [P, DC, D] : M = Wq^T Wk chunked — mm[p, c, e] = M[c*P + p, e]
    #   wv       [P, DC, D] : Wv^T chunked — wv[p, c, e] = Wv[e, c*P + p]
    #   uu       [P, DC]    : u = Wk^T bq — uu[p, c] = u[c*P + p]
    #   bv       [1, D]
    #   outp     [P, N//P, D] : out[p, blk, d] = y[blk*P + p, d]
    xq = nc.dram_tensor("xq", [P, DC, N], fp16, kind="ExternalInput").ap()
    xk = nc.dram_tensor("xk", [P, DC, N], fp16, kind="ExternalInput").ap()
    xv = nc.dram_tensor("xv", [P, DC, N], bf16, kind="ExternalInput").ap()
    mm = nc.dram_tensor("mm", [P, DC, D], fp16, kind="ExternalInput").ap()
    wv = nc.dram_tensor("wv", [P, DC, D], bf16, kind="ExternalInput").ap()
    uu = nc.dram_tensor("uu", [P, DC], fp16, kind="ExternalInput").ap()
    bv = nc.dram_tensor("bv", [1, D], f32, kind="ExternalInput").ap()

    outp = nc.dram_tensor("outp", [P, N // P, D], bf16, kind="ExternalOutput").ap()

    HN = N // NH  # tokens per x half

    with ExitStack() as ctx:
        tc = ctx.enter_context(tile.TileContext(nc))
        res = ctx.enter_context(tc.tile_pool(name="res", bufs=1))
        xst = ctx.enter_context(tc.tile_pool(name="xst", bufs=2))
        wst = ctx.enter_context(tc.tile_pool(name="wst", bufs=2))
        exp_ = ctx.enter_context(tc.tile_pool(name="exp", bufs=1))
        outs = ctx.enter_context(tc.tile_pool(name="outs", bufs=2))
        sml = ctx.enter_context(tc.tile_pool(name="sml", bufs=1))
        smr = ctx.enter_context(tc.tile_pool(name="smr", bufs=2))
        psa = ctx.enter_context(tc.tile_pool(name="psa", bufs=4, space="PSUM"))
        pss = ctx.enter_context(tc.tile_pool(name="pss", bufs=3, space="PSUM"))
        psr = ctx.enter_context(tc.tile_pool(name="psr", bufs=1, space="PSUM"))

        # ---- constants / small tensors ----
        ones_b = sml.tile([P, 1], bf16, name="ones_b", tag="c_ones")
        nc.vector.memset(ones_b, 1.0)
        u_sb = sml.tile([P, DC], fp16, name="u_sb", tag="c_u")
        nc.sync.dma_start(out=u_sb, in_=uu)
        bv_sb = sml.tile([1, D], f32, name="bv_sb", tag="c_bv")
        nc.sync.dma_start(out=bv_sb, in_=bv)
        ones_row = sml.tile([1, P], f32, name="ones_row", tag="c_onesr")
        nc.vector.memset(ones_row, 1.0)
        # bv broadcast across partitions: bvb[p, e] = bv[e]
        bv_bc = sml.tile([P, D], f32, name="bv_bc", tag="c_bvb")
        for h in range(2):
            pb = psa.tile([P, F], f32, name="pb", tag="psa")
            nc.tensor.matmul(
                pb, lhsT=ones_row, rhs=bv_sb[:, h * F:(h + 1) * F],
                start=True, stop=True,
            )
            nc.vector.tensor_copy(bv_bc[:, h * F:(h + 1) * F], pb)

        # ---- resident tiles ----
        xkr = res.tile([P, DC, N], fp16, name="xkr", tag="kt")    # xk^T[d, n]
        vv = res.tile([P, JT, D], bf16, name="vv", tag="vv")      # v'[n, e]
        t1 = res.tile([P, DC, N], fp16, name="t1", tag="qt")      # (xq M)^T[d', n]
        c_sb = sml.tile([P, JT], f32, name="c_sb", tag="c_c")     # c_j = u.xk_j

        # ---- Phase 1: t1 = (xq M)^T, xk resident + c vector, v' ----
        # DMA issue order is tuned so the first-needed bytes (M, first xq
        # quarter) are not queued behind the 4 MB xk load; xk/wv/xv stream
        # in under the t1 compute.
        def t1_compute(w_sb, xh, h):
            for c in range(DC):
                for nb in range(HN // F):
                    ps = psa.tile([P, F], f32, name="ps_p", tag="psa")
                    for dp in range(DC):
                        nc.tensor.matmul(
                            ps,
                            lhsT=w_sb[:, dp, c * P:(c + 1) * P],
                            rhs=xh[:, dp, nb * F:(nb + 1) * F],
                            start=(dp == 0),
                            stop=(dp == DC - 1),
                        )
                    nc.scalar.copy(
                        t1[:, c, h * HN + nb * F:h * HN + (nb + 1) * F], ps
                    )

        def load_x(xsrc, h, dt):
            xh = xst.tile([P, DC, HN], dt, name="xh", tag="xj")
            for q4 in range(2):
                nc.sync.dma_start(
                    out=xh[:, :, q4 * (HN // 2):(q4 + 1) * (HN // 2)],
                    in_=xsrc[:, :, h * HN + q4 * (HN // 2):h * HN + (q4 + 1) * (HN // 2)],
                )
            return xh

        m_sb = wst.tile([P, DC, D], fp16, name="m_sb", tag="w")
        nc.sync.dma_start(out=m_sb, in_=mm)
        xq0 = load_x(xq, 0, fp16)
        t1_compute(m_sb, xq0, 0)
        # stream the rest in under the t1 h0 compute
        for q4 in range(4):
            nc.sync.dma_start(
                out=xkr[:, :, q4 * (N // 4):(q4 + 1) * (N // 4)],
                in_=xk[:, :, q4 * (N // 4):(q4 + 1) * (N // 4)],
            )
        xq1 = load_x(xq, 1, fp16)
        wv_sb = wst.tile([P, DC, D], bf16, name="wv_sb", tag="w")
        nc.sync.dma_start(out=wv_sb, in_=wv)
        xv0 = load_x(xv, 0, bf16)
        t1_compute(m_sb, xq1, 1)
        xv1 = load_x(xv, 1, bf16)

        # c_j = u . xk_j, per-key bias, one psum column per key tile
        c_ps = psr.tile([P, JT], f32, name="c_ps", tag="psr")
        for jt in range(JT):
            for dp in range(DC):
                nc.tensor.matmul(
                    c_ps[:, jt:jt + 1],
                    lhsT=xkr[:, dp, jt * P:(jt + 1) * P],
                    rhs=u_sb[:, dp:dp + 1],
                    start=(dp == 0),
                    stop=(dp == DC - 1),
                    skip_group_check=True,
                )
        nc.vector.tensor_copy(c_sb, c_ps)

        for h in range(NH):
            xh = xv0 if h == 0 else xv1
            for jt in range(HN // P):
                jg = h * (HN // P) + jt
                for eh in range(D // F):
                    ps = psa.tile([P, F], f32, name="ps_v", tag="psa")
                    for dp in range(DC):
                        nc.tensor.matmul(
                            ps,
                            lhsT=xh[:, dp, jt * P:(jt + 1) * P],
                            rhs=wv_sb[:, dp, eh * F:(eh + 1) * F],
                            start=(dp == 0),
                            stop=(dp == DC - 1),
                        )
                    nc.vector.tensor_tensor(
                        out=vv[:, jg, eh * F:(eh + 1) * F],
                        in0=ps,
                        in1=bv_bc[:, eh * F:(eh + 1) * F],
                        op=mybir.AluOpType.add,
                    )

        # ---- Phase 2: per 512-query block: scores^T, exp, rowsum, PV ----
        for t in range(NB):
            ex = exp_.tile([P, JT, F], bf16, name="ex", tag="ex")
            for jt in range(JT):
                ps = pss.tile([P, F], f32, name="ps_s", tag="pss")
                for c in range(DC):
                    nc.tensor.matmul(
                        ps,
                        lhsT=xkr[:, c, jt * P:(jt + 1) * P],
                        rhs=t1[:, c, t * F:(t + 1) * F],
                        start=(c == 0),
                        stop=(c == DC - 1),
                    )
                nc.scalar.activation(ex[:, jt, :], ps, EXP, bias=c_sb[:, jt:jt + 1])

            # PV with ex as the stationary operand: psum comes out directly in
            # natural [q, d] layout, so normalization is a single
            # per-partition tensor_scalar and no transposes are needed. The
            # per-partition rowsum [128,1] falls out of the same stationary
            # operand against a ones vector (ap_size-1 matmuls, ~free).
            ot = outs.tile([P, NB, D], bf16, name="ot", tag="ot")
            for qb in range(NB):
                rps = psr.tile([P, 1], f32, name="rps", tag="psr")
                pv0 = psa.tile([P, F], f32, name="pv0", tag="psa")
                pv1 = psa.tile([P, F], f32, name="pv1", tag="psa")
                # rowsum + both PV halves interleaved per jt: three back-to-
                # back matmuls share the same stationary ex tile, so the PE
                # loads each weight tile once per qb instead of three times.
                for jt in range(JT):
                    exs = ex[:, jt, qb * P:(qb + 1) * P]
                    nc.tensor.matmul(
                        rps, lhsT=exs, rhs=ones_b,
                        start=(jt == 0), stop=(jt == JT - 1),
                        skip_group_check=True,
                    )
                    nc.tensor.matmul(
                        pv0, lhsT=exs, rhs=vv[:, jt, 0:F],
                        start=(jt == 0), stop=(jt == JT - 1),
                        skip_group_check=True,
                    )
                    nc.tensor.matmul(
                        pv1, lhsT=exs, rhs=vv[:, jt, F:D],
                        start=(jt == 0), stop=(jt == JT - 1),
                        skip_group_check=True,
                    )
                rinv = smr.tile([P, 1], f32, name="rinv", tag="rinv")
                nc.vector.reciprocal(rinv, rps)
                nc.vector.tensor_scalar_mul(ot[:, qb, 0:F], pv0, rinv)
                nc.vector.tensor_scalar_mul(ot[:, qb, F:D], pv1, rinv)
                nc.sync.dma_start(
                    out=outp[:, t * NB + qb:t * NB + qb + 1, :], in_=ot[:, qb:qb + 1, :]
                )

    nc.compile()
    return nc


def get_nc():
    if "nc" not in _CACHE:
        _CACHE["nc"] = _build_nc()
    return _CACHE["nc"]


def make_in_maps(query, key, value, Wq, bq, Wk, bk, Wv, bv):
    import ml_dtypes

    bf16 = ml_dtypes.bfloat16
    query = np.asarray(query, dtype=np.float32)
    key = np.asarray(key, dtype=np.float32)
    value = np.asarray(value, dtype=np.float32)

    def pack_w(W, dt):
        # [P, DC, D]: w[p, c, e] = W[e, c*P + p]
        Wt = np.asarray(W, dtype=np.float32).T  # [d, e]
        return np.ascontiguousarray(
            Wt.reshape(DC, P, D).transpose(1, 0, 2)
        ).astype(dt)

    def pack_x(x, dt):
        # [P, DC, N]: xp[p, c, n] = x[n, c*P + p]
        return np.ascontiguousarray(
            x.reshape(N, DC, P).transpose(2, 1, 0)
        ).astype(dt)

    Wq32 = np.asarray(Wq, dtype=np.float32)
    Wk32 = np.asarray(Wk, dtype=np.float32)
    bq32 = np.asarray(bq, dtype=np.float32)
    M = Wq32.T @ Wk32                     # [d, d']: logits = xq M xk^T + ...
    u = Wk32.T @ bq32                     # [d']:   ... + (u . xk_j)
    m_p = np.ascontiguousarray(
        M.reshape(DC, P, D).transpose(1, 0, 2)
    ).astype(np.float16)
    wv_p = pack_w(Wv, bf16)
    u_p = np.ascontiguousarray(u.reshape(DC, P).T).astype(np.float16)
    bv_p = np.ascontiguousarray(np.asarray(bv, dtype=np.float32).reshape(1, D))

    in_maps = []
    for b in range(B):
        in_maps.append(
            {
                "xq": pack_x(query[b], np.float16),
                "xk": pack_x(key[b], np.float16),
                "xv": pack_x(value[b], bf16),
                "mm": m_p,
                "wv": wv_p,
                "uu": u_p,
                "bv": bv_p,
            }
        )
    return in_maps


def postprocess(results):
    outs = []
    for b in range(B):
        op = np.asarray(results[b]["outp"])  # [P, N//P, D] bf16
        outs.append(
            op.astype(np.float32).transpose(1, 0, 2).reshape(N, D)
        )
    return np.stack(outs)


def kernel(query, key, value, Wq, bq, Wk, bk, Wv, bv):
    from concourse.bass_utils import run_bass_kernel_spmd

    nc = get_nc()
    in_maps = make_in_maps(query, key, value, Wq, bq, Wk, bk, Wv, bv)
    res = run_bass_kernel_spmd(nc, in_maps, list(range(B)))
    return postprocess(res.results)

